# revision 17
# baseline (speedup 1.0000x reference)
import math

import numpy as np

import concourse.tile as tile
from concourse import bacc, mybir
from concourse.bass_utils import run_bass_kernel_spmd

L, D_IN, D_HID, D_OUT, NTOT = 8, 256, 1024, 256, 32768
BLKS = [512] * 8 + [128]
COFF = [sum(BLKS[:i]) for i in range(len(BLKS) + 1)]
NBLK = len(BLKS)
P = COFF[-1]

KI = D_IN // 128
MJ = D_HID // 128
MO = D_OUT // 128

F32 = mybir.dt.float32
F32R = mybir.dt.float32r
AF = mybir.ActivationFunctionType

PROFILE = False
LAST_RES = None

_nc_cache = None
_erf = np.vectorize(math.erf)


def _build_nc():
    nc = bacc.Bacc()
    xp_d = nc.declare_dram_parameter("xp", [128, KI * P], F32R, isOutput=False)
    w1p_d = nc.declare_dram_parameter("w1p", [128, MJ * KI * 128], F32R, isOutput=False)
    w2p_d = nc.declare_dram_parameter("w2p", [128, MJ * MO * 128], F32R, isOutput=False)
    b1p_d = nc.declare_dram_parameter("b1p", [128, MJ], F32, isOutput=False)
    b2p_d = nc.declare_dram_parameter("b2p", [128, MO], F32, isOutput=False)
    outp_d = nc.declare_dram_parameter("outp", [128, MO * P], F32, isOutput=True)

    def x_load(pool, ib):
        w = BLKS[ib]
        ts = []
        for k in range(KI):
            r = pool.tile([128, w], F32R, tag=f"x{k}")
            o = KI * COFF[ib] + k * w
            nc.gpsimd.dma_start(r[:], xp_d[:, o:o + w])
            ts.append(r)
        return ts

    with tile.TileContext(nc) as tc:
        with (
            tc.tile_pool(name="wpool", bufs=1) as wp,
            tc.tile_pool(name="xr", bufs=3) as xrp,
            tc.tile_pool(name="hr", bufs=2) as hrp,
            tc.tile_pool(name="outp", bufs=2) as outp,
            tc.tile_pool(name="ps1", bufs=6, space="PSUM") as ps1,
            tc.tile_pool(name="ps2", bufs=1, space="PSUM") as ps2,
        ):
            w1r = []
            t = wp.tile([128, KI * 128], F32R, tag="w1j0")
            nc.scalar.dma_start(t[:], w1p_d[:, 0:KI * 128])
            w1r.append(t)
            b1_t = wp.tile([128, MJ], F32, tag="b1")
            nc.scalar.dma_start(b1_t[:], b1p_d[:])
            xcur = x_load(xrp, 0)
            HW = MJ * 128
            w2h = []
            for i in range(MO):
                t2 = wp.tile([128, HW], F32R, tag=f"w2i{i}")
                nc.sync.dma_start(t2[:], w2p_d[:, i * HW:(i + 1) * HW])
                w2h.append(t2)
            b2_t = wp.tile([128, MO], F32, tag="b2")
            nc.sync.dma_start(b2_t[:], b2p_d[:])
            for j in range(1, MJ):
                t = wp.tile([128, KI * 128], F32R, tag=f"w1j{j}")
                nc.scalar.dma_start(t[:], w1p_d[:, j * KI * 128:(j + 1) * KI * 128])
                w1r.append(t)

            for ib in range(NBLK):
                w = BLKS[ib]
                xr = xcur
                if ib + 1 < NBLK:
                    xcur = x_load(xrp, ib + 1)
                hr = []
                for j in range(MJ):
                    pt = ps1.tile([128, w], F32, tag="h")
                    for k in range(KI):
                        nc.tensor.matmul(pt[:], w1r[j][:, k * 128:(k + 1) * 128],
                                         xr[k][:], start=(k == 0), stop=(k == KI - 1))
                    h = hrp.tile([128, w], F32R, tag=f"h{j}")
                    nc.scalar.activation(h[:], pt[:], AF.Gelu, bias=b1_t[:, j:j + 1])
                    hr.append(h)
                ot = outp.tile([128, MO * w], F32, tag="o")
                for i in range(MO):
                    pt = ps2.tile([128, w], F32, tag=f"o{i}")
                    for j in range(MJ):
                        nc.tensor.matmul(pt[:], w2h[i][:, j * 128:(j + 1) * 128],
                                         hr[j][:], start=(j == 0), stop=(j == MJ - 1))
                    nc.vector.tensor_scalar_add(ot[:, i * w:(i + 1) * w], pt[:],
                                                b2_t[:, i:i + 1])
                nc.sync.dma_start(
                    outp_d[:, MO * COFF[ib]:MO * COFF[ib + 1]], ot[:])
    if not nc.is_finalized():
        nc.finalize()
    return nc


def _pack_x(xt):
    xp = np.empty((128, KI * P), dtype=np.float32)
    for ib in range(NBLK):
        w = BLKS[ib]
        for k in range(KI):
            o = KI * COFF[ib] + k * w
            xp[:, o:o + w] = xt[k * 128:(k + 1) * 128, COFF[ib]:COFF[ib] + w]
    return xp


def _unpack_out(op):
    outT = np.empty((D_OUT, P), dtype=np.float32)
    for ib in range(NBLK):
        w = BLKS[ib]
        for i in range(MO):
            o = MO * COFF[ib] + i * w
            outT[i * 128:(i + 1) * 128, COFF[ib]:COFF[ib] + w] = op[:, o:o + w]
    return outT


def kernel(x, W1, b1, W2, b2, plane_idx):
    global _nc_cache, LAST_RES
    x = np.ascontiguousarray(x, dtype=np.float32)
    W1 = np.asarray(W1, dtype=np.float32)
    b1 = np.asarray(b1, dtype=np.float32)
    W2 = np.asarray(W2, dtype=np.float32)
    b2 = np.asarray(b2, dtype=np.float32)
    plane_idx = np.asarray(plane_idx)

    order = np.argsort(plane_idx, kind="stable")
    counts = np.bincount(plane_idx, minlength=L)
    starts = np.concatenate([[0], np.cumsum(counts)])

    in_maps = []
    idxs = []
    for c in range(L):
        idx = order[starts[c]:starts[c + 1]]
        idxs.append(idx)
        n = min(len(idx), P)
        xt = np.zeros((D_IN, P), dtype=np.float32)
        xt[:, :n] = x[idx[:n]].T
        w1p = np.ascontiguousarray(
            W1[c].T.reshape(KI, 128, MJ, 128).transpose(1, 2, 0, 3).reshape(128, MJ * KI * 128))
        w2p = np.ascontiguousarray(
            W2[c].T.reshape(MJ, 128, MO, 128).transpose(1, 2, 0, 3).reshape(128, MO * MJ * 128))
        in_maps.append({
            "xp": _pack_x(xt),
            "w1p": w1p,
            "w2p": w2p,
            "b1p": np.ascontiguousarray(b1[c].reshape(MJ, 128).T),
            "b2p": np.ascontiguousarray(b2[c].reshape(MO, 128).T),
        })

    if _nc_cache is None:
        _nc_cache = _build_nc()
    res = run_bass_kernel_spmd(_nc_cache, in_maps, list(range(L)), trace=PROFILE)
    LAST_RES = res

    out = np.empty((NTOT, D_OUT), dtype=np.float32)
    for c in range(L):
        idx = idxs[c]
        n = min(len(idx), P)
        outT = _unpack_out(res.results[c]["outp"])
        out[idx[:n]] = outT[:, :n].T
        if len(idx) > n:
            xo = x[idx[n:]].astype(np.float64)
            h = xo @ W1[c].T.astype(np.float64) + b1[c].astype(np.float64)
            g = 0.5 * h * (1.0 + _erf(h / math.sqrt(2.0)).astype(np.float64))
            o = g @ W2[c].T.astype(np.float64) + b2[c].astype(np.float64)
            out[idx[n:]] = o.astype(np.float32)
    return out
